# revision 23
# baseline (speedup 1.0000x reference)
"""Causal depthwise-conv self-attention kernel for Trainium2 (8 NeuronCores).

Math: out[b,t,d] = sum_i sum_k X[b,t-i,k] * W[i*D+d,k]   (i in 0..kW-1, zero for t<i)

Sharding: 8 cores = 2 batches x 4 channel-groups (256 output channels each).
Each core computes out^T[c, t] = sum_{kc,i} Wt[:,kc,i,c].T @ XT[:,kc,t-i] with
the tap shift expressed as a free-dim offset into a zero-padded X^T tile
resident in SBUF. fp32r matmuls (full PE rate, ~1e-4 rel precision).
Host does the X transpose / W reshape on the way in and the output
transpose on the way out; no on-device collectives.
"""

import numpy as np

import concourse.bacc as bacc
import concourse.mybir as mybir
import concourse.tile as tile
from concourse.bass_utils import run_bass_kernel_spmd

# bass_utils imports antenv.axon_hooks when BASS_TRACE is set; that module is
# absent from this image. Provide a no-op stand-in so tracing degrades
# gracefully instead of crashing the run.
try:
    import antenv.axon_hooks  # noqa: F401
except ImportError:
    import sys
    import types

    import antenv

    _hooks = types.ModuleType("antenv.axon_hooks")
    _hooks._h = None
    _hooks.set_axon_ntff_profile_hook = lambda h: setattr(_hooks, "_h", h)
    _hooks.get_axon_ntff_profile_hook = lambda: _hooks._h
    sys.modules["antenv.axon_hooks"] = _hooks
    antenv.axon_hooks = _hooks

BSZ, T, D, KW = 2, 4096, 1024, 4
NCORES = 8
CGROUPS = 4            # channel groups (one per core within a batch)
CPG = D // CGROUPS     # channels per core = 256
KC = D // 128          # contraction chunks = 8
TT = T // 512          # t tiles of 512 = 8
PAD = KW - 1           # causal halo columns = 3
CS = CPG // 128        # channel subtiles per core = 2
WARMUP_MMS = 12        # PE busy-burst during initial DMA (flips HAM to 8/8)

_last_results = None   # test harness peeks at this for profiling info
_nc_cache = None       # compiled program reused across kernel() calls


def _build_nc():
    nc = bacc.Bacc(trn_type="TRN2", enable_partition_id=False)
    xt = nc.dram_tensor("xt", [128, KC, PAD + T], mybir.dt.float32r,
                        kind="ExternalInput")
    wt = nc.dram_tensor("wt", [128, KC, KW, CPG], mybir.dt.float32r,
                        kind="ExternalInput")
    out_ct = nc.dram_tensor("out_ct", [CS, 128, T], mybir.dt.float32,
                            kind="ExternalOutput")

    with tile.TileContext(nc) as tc:
        with (
            tc.tile_pool(name="xpool", bufs=1) as xpool,
            tc.tile_pool(name="wpool", bufs=1) as wpool,
            tc.tile_pool(name="opool", bufs=6) as opool,
            tc.tile_pool(name="psum", bufs=8, space="PSUM") as psum_pool,
        ):
            xt_sb = xpool.tile([128, KC, PAD + T], mybir.dt.float32r)
            wt_sb = wpool.tile([128, KC, KW, CPG], mybir.dt.float32r)
            dummy = wpool.tile([128, 512], mybir.dt.float32r, name="dummy")
            nc.gpsimd.memset(dummy[:].bitcast(mybir.dt.float32), 0.0)

            # Issue DMAs in first-needed order. Time axis is processed in two
            # phases (t-halves); phase A only needs xt cols [0:HALF] plus the
            # weights, so early-kernel DMA demand is ~halved and the PE never
            # outruns HBM even under 8-core contention.
            HALF = 2051  # covers rhs windows of t-tiles 0..3 (incl. halo)
            QTR = 1027   # covers rhs windows of t-tiles 0..1 (incl. halo)
            # weights ride the Scalar HWDGE ring, X^T the Sync ring, so the
            # first matmul's two dependencies stream concurrently
            for kc in range(KC):
                nc.scalar.dma_start(wt_sb[:, kc], wt[:, kc])
            nc.sync.dma_start(xt_sb[:, 0, :QTR], xt[:, 0, :QTR])
            nc.sync.dma_start(xt_sb[:, 0, QTR:HALF], xt[:, 0, QTR:HALF])
            for kc in range(1, KC):
                nc.sync.dma_start(xt_sb[:, kc, :HALF], xt[:, kc, :HALF])
            for kc in range(KC):
                nc.sync.dma_start(xt_sb[:, kc, HALF:], xt[:, kc, HALF:])

            # HAM warmup: keep PE busy while the first DMAs land.
            ps_w = psum_pool.tile([128, 512], mybir.dt.float32,
                                  name="ps_warm", tag="ps")
            for w in range(WARMUP_MMS):
                nc.tensor.matmul(ps_w[:], dummy[:, :128], dummy[:],
                                 start=True, stop=True, skip_group_check=True)

            HT = TT // 2  # t tiles per phase
            for half in range(2):
                psums = {}
                for cs in range(CS):
                    for tj2 in range(HT):
                        psums[cs, tj2] = psum_pool.tile(
                            [128, 512], mybir.dt.float32,
                            name=f"ps_{half}_{cs}_{tj2}", tag="ps")
                for kc in range(KC):
                    if half == 0 and kc == 0:
                        # first k-chunk: consume the quarter-split DMAs in order
                        order = [(cs, i, tj2) for q in range(2)
                                 for cs in range(CS) for i in range(KW)
                                 for tj2 in (2 * q, 2 * q + 1)]
                    elif kc < KC - 1:
                        order = [(cs, i, tj2) for cs in range(CS)
                                 for i in range(KW) for tj2 in range(HT)]
                    else:
                        # last k-chunk: finish PSUM tiles staggered so
                        # copyback/DMA-out overlap the remaining matmuls
                        order = [(cs, i, tj2) for tj2 in range(HT)
                                 for cs in range(CS) for i in range(KW)]
                    for cs, i, tj2 in order:
                        tj = half * HT + tj2
                        lo = PAD + tj * 512 - i
                        nc.tensor.matmul(
                            psums[cs, tj2][:],
                            wt_sb[:, kc, i, cs * 128:(cs + 1) * 128],
                            xt_sb[:, kc, lo:lo + 512],
                            start=(kc == 0 and i == 0),
                            stop=(kc == KC - 1 and i == KW - 1),
                        )
                for n, (cs, tj2) in enumerate(
                        [(cs, tj2) for tj2 in range(HT) for cs in range(CS)]):
                    tj = half * HT + tj2
                    o = opool.tile([128, 512], mybir.dt.float32,
                                   name=f"o_{half}_{cs}_{tj2}", tag="obuf")
                    dst = out_ct[cs, :, tj * 512:(tj + 1) * 512]
                    if n == CS * HT - 1:
                        # last-finishing tile: pipeline copy->DMA in halves so
                        # the final DMA flush covers half the bytes
                        nc.vector.tensor_copy(out=o[:, :256],
                                              in_=psums[cs, tj2][:, :256])
                        nc.sync.dma_start(dst[:, :256], o[:, :256])
                        nc.vector.tensor_copy(out=o[:, 256:],
                                              in_=psums[cs, tj2][:, 256:])
                        nc.sync.dma_start(dst[:, 256:], o[:, 256:])
                    elif n % 2 == 0:
                        nc.scalar.copy(o[:], psums[cs, tj2][:])
                        nc.sync.dma_start(dst, o[:])
                    else:
                        nc.vector.tensor_copy(out=o[:], in_=psums[cs, tj2][:])
                        nc.sync.dma_start(dst, o[:])

    nc.compile()
    return nc


def kernel(X: np.ndarray, W: np.ndarray) -> np.ndarray:
    global _last_results
    X = np.ascontiguousarray(X, dtype=np.float32)
    W = np.ascontiguousarray(W, dtype=np.float32)

    # X^T per batch with causal zero-halo: xt[p, kc, PAD+t] = X[b, t, kc*128+p]
    xts = []
    for b in range(BSZ):
        xt = np.zeros((128, KC, PAD + T), dtype=np.float32)
        xt[:, :, PAD:] = X[b].reshape(T, KC, 128).transpose(2, 1, 0)
        xts.append(xt)

    # W per core: wt[p, kc, i, c] = W[i*D + cg*CPG + c, kc*128 + p]
    W4 = W.reshape(KW, D, KC, 128)  # [i, d, kc, p]
    wts = []
    for cg in range(CGROUPS):
        wt = W4[:, cg * CPG:(cg + 1) * CPG, :, :].transpose(3, 2, 0, 1)
        wts.append(np.ascontiguousarray(wt))

    global _nc_cache
    if _nc_cache is None:
        _nc_cache = _build_nc()
    nc = _nc_cache
    in_maps = [{"xt": xts[c // CGROUPS], "wt": wts[c % CGROUPS]}
               for c in range(NCORES)]
    _last_results = run_bass_kernel_spmd(nc, in_maps, core_ids=list(range(NCORES)))

    out = np.empty((BSZ, T, D), dtype=np.float32)
    for c in range(NCORES):
        b, cg = c // CGROUPS, c % CGROUPS
        shard = _last_results.results[c]["out_ct"].reshape(CPG, T)
        out[b, :, cg * CPG:(cg + 1) * CPG] = shard.T
    return out


# revision 24
# speedup vs baseline: 1.0041x; 1.0041x over previous
"""Causal depthwise-conv self-attention kernel for Trainium2 (8 NeuronCores).

Math: out[b,t,d] = sum_i sum_k X[b,t-i,k] * W[i*D+d,k]   (i in 0..kW-1, zero for t<i)

Sharding: 8 cores = 2 batches x 4 channel-groups (256 output channels each).
Each core computes out^T[c, t] = sum_{kc,i} Wt[:,kc,i,c].T @ XT[:,kc,t-i] with
the tap shift expressed as a free-dim offset into a zero-padded X^T tile
resident in SBUF. fp32r matmuls (full PE rate, ~1e-4 rel precision).
Host does the X transpose / W reshape on the way in and the output
transpose on the way out; no on-device collectives.
"""

import numpy as np

import concourse.bacc as bacc
import concourse.mybir as mybir
import concourse.tile as tile
from concourse.bass_utils import run_bass_kernel_spmd

# bass_utils imports antenv.axon_hooks when BASS_TRACE is set; that module is
# absent from this image. Provide a no-op stand-in so tracing degrades
# gracefully instead of crashing the run.
try:
    import antenv.axon_hooks  # noqa: F401
except ImportError:
    import sys
    import types

    import antenv

    _hooks = types.ModuleType("antenv.axon_hooks")
    _hooks._h = None
    _hooks.set_axon_ntff_profile_hook = lambda h: setattr(_hooks, "_h", h)
    _hooks.get_axon_ntff_profile_hook = lambda: _hooks._h
    sys.modules["antenv.axon_hooks"] = _hooks
    antenv.axon_hooks = _hooks

BSZ, T, D, KW = 2, 4096, 1024, 4
NCORES = 8
CGROUPS = 4            # channel groups (one per core within a batch)
CPG = D // CGROUPS     # channels per core = 256
KC = D // 128          # contraction chunks = 8
TT = T // 512          # t tiles of 512 = 8
PAD = KW - 1           # causal halo columns = 3
CS = CPG // 128        # channel subtiles per core = 2
WARMUP_MMS = 12        # PE busy-burst during initial DMA (flips HAM to 8/8)

_last_results = None   # test harness peeks at this for profiling info
_nc_cache = None       # compiled program reused across kernel() calls


def _build_nc():
    nc = bacc.Bacc(trn_type="TRN2", enable_partition_id=False)
    xt = nc.dram_tensor("xt", [128, KC, PAD + T], mybir.dt.float32r,
                        kind="ExternalInput")
    wt = nc.dram_tensor("wt", [128, KC, KW, CPG], mybir.dt.float32r,
                        kind="ExternalInput")
    out_ct = nc.dram_tensor("out_ct", [CS, 128, T], mybir.dt.float32,
                            kind="ExternalOutput")

    with tile.TileContext(nc) as tc:
        with (
            tc.tile_pool(name="xpool", bufs=1) as xpool,
            tc.tile_pool(name="wpool", bufs=1) as wpool,
            tc.tile_pool(name="opool", bufs=6) as opool,
            tc.tile_pool(name="psum", bufs=8, space="PSUM") as psum_pool,
        ):
            xt_sb = xpool.tile([128, KC, PAD + T], mybir.dt.float32r)
            wt_sb = wpool.tile([128, KC, KW, CPG], mybir.dt.float32r)
            dummy = wpool.tile([128, 512], mybir.dt.float32r, name="dummy")
            nc.gpsimd.memset(dummy[:].bitcast(mybir.dt.float32), 0.0)

            # Issue DMAs in first-needed order. Time axis is processed in two
            # phases (t-halves); phase A only needs xt cols [0:HALF] plus the
            # weights, so early-kernel DMA demand is ~halved and the PE never
            # outruns HBM even under 8-core contention.
            HALF = 2051  # covers rhs windows of t-tiles 0..3 (incl. halo)
            QTR = 1027   # covers rhs windows of t-tiles 0..1 (incl. halo)
            # weights ride the Scalar HWDGE ring, X^T the Sync ring, so the
            # first matmul's two dependencies stream concurrently
            for kc in range(KC):
                nc.scalar.dma_start(wt_sb[:, kc], wt[:, kc])
            nc.sync.dma_start(xt_sb[:, 0, :QTR], xt[:, 0, :QTR])
            nc.sync.dma_start(xt_sb[:, 0, QTR:HALF], xt[:, 0, QTR:HALF])
            for kc in range(1, KC):
                nc.sync.dma_start(xt_sb[:, kc, :HALF], xt[:, kc, :HALF])
            for kc in range(KC):
                nc.sync.dma_start(xt_sb[:, kc, HALF:], xt[:, kc, HALF:])

            # HAM warmup: keep PE busy while the first DMAs land.
            ps_w = psum_pool.tile([128, 512], mybir.dt.float32,
                                  name="ps_warm", tag="ps")
            for w in range(WARMUP_MMS):
                nc.tensor.matmul(ps_w[:], dummy[:, :128], dummy[:],
                                 start=True, stop=True, skip_group_check=True)

            HT = TT // 2  # t tiles per phase
            for half in range(2):
                psums = {}
                for cs in range(CS):
                    for tj2 in range(HT):
                        psums[cs, tj2] = psum_pool.tile(
                            [128, 512], mybir.dt.float32,
                            name=f"ps_{half}_{cs}_{tj2}", tag="ps")
                for kc in range(KC):
                    if half == 0 and kc == 0:
                        # first k-chunk: consume the quarter-split DMAs in order
                        order = [(cs, i, tj2) for q in range(2)
                                 for cs in range(CS) for i in range(KW)
                                 for tj2 in (2 * q, 2 * q + 1)]
                    elif kc < KC - 1:
                        order = [(cs, i, tj2) for cs in range(CS)
                                 for i in range(KW) for tj2 in range(HT)]
                    else:
                        # last k-chunk: finish PSUM tiles staggered so
                        # copyback/DMA-out overlap the remaining matmuls
                        order = [(cs, i, tj2) for tj2 in range(HT)
                                 for cs in range(CS) for i in range(KW)]
                    for cs, i, tj2 in order:
                        tj = half * HT + tj2
                        lo = PAD + tj * 512 - i
                        nc.tensor.matmul(
                            psums[cs, tj2][:],
                            wt_sb[:, kc, i, cs * 128:(cs + 1) * 128],
                            xt_sb[:, kc, lo:lo + 512],
                            start=(kc == 0 and i == 0),
                            stop=(kc == KC - 1 and i == KW - 1),
                        )
                for n, (cs, tj2) in enumerate(
                        [(cs, tj2) for tj2 in range(HT) for cs in range(CS)]):
                    tj = half * HT + tj2
                    o = opool.tile([128, 512], mybir.dt.float32,
                                   name=f"o_{half}_{cs}_{tj2}", tag="obuf")
                    if n % 2 == 0:
                        nc.scalar.copy(o[:], psums[cs, tj2][:])
                    else:
                        nc.vector.tensor_copy(out=o[:], in_=psums[cs, tj2][:])
                    nc.sync.dma_start(out_ct[cs, :, tj * 512:(tj + 1) * 512], o[:])

    nc.compile()
    return nc


def kernel(X: np.ndarray, W: np.ndarray) -> np.ndarray:
    global _last_results
    X = np.ascontiguousarray(X, dtype=np.float32)
    W = np.ascontiguousarray(W, dtype=np.float32)

    # X^T per batch with causal zero-halo: xt[p, kc, PAD+t] = X[b, t, kc*128+p]
    xts = []
    for b in range(BSZ):
        xt = np.zeros((128, KC, PAD + T), dtype=np.float32)
        xt[:, :, PAD:] = X[b].reshape(T, KC, 128).transpose(2, 1, 0)
        xts.append(xt)

    # W per core: wt[p, kc, i, c] = W[i*D + cg*CPG + c, kc*128 + p]
    W4 = W.reshape(KW, D, KC, 128)  # [i, d, kc, p]
    wts = []
    for cg in range(CGROUPS):
        wt = W4[:, cg * CPG:(cg + 1) * CPG, :, :].transpose(3, 2, 0, 1)
        wts.append(np.ascontiguousarray(wt))

    global _nc_cache
    if _nc_cache is None:
        _nc_cache = _build_nc()
    nc = _nc_cache
    in_maps = [{"xt": xts[c // CGROUPS], "wt": wts[c % CGROUPS]}
               for c in range(NCORES)]
    _last_results = run_bass_kernel_spmd(nc, in_maps, core_ids=list(range(NCORES)))

    out = np.empty((BSZ, T, D), dtype=np.float32)
    for c in range(NCORES):
        b, cg = c // CGROUPS, c % CGROUPS
        shard = _last_results.results[c]["out_ct"].reshape(CPG, T)
        out[b, :, cg * CPG:(cg + 1) * CPG] = shard.T
    return out
